# revision 20
# baseline (speedup 1.0000x reference)
"""Causal self-attention (B=2, T=2048, C=1024, H=16, D=64) on 8 trn2 cores.

Sharding: core c -> batch b = c // 4, head-group g = c % 4 (4 heads each).
Data-parallel over B, tensor-parallel (Megatron) over heads for the
qkv / proj linears. Each core computes its head-group's attention and a
partial output projection; the host sums the 4 partials per batch and
adds the proj bias.

Everything on-device is done in transposed [feature, token] space:
  qk^T = Wqk @ x^T                       (PE; bias added by ACT on evacuate)
  v    = x @ Wv^T (+ ones column)        (PE; K=1 matmul adds bias + ones)
  att^T[k, q] = k^T(head)^T . q(head)    (PE, K=64; causal tiles skipped)
  P = exp(att^T + additive causal mask)  (ACT; no max-subtraction needed,
                                          |logits| <~ 10 so fp32 exp is safe)
  rawout^T[d+1, q] = v_aug^T @ P         (PE accumulate over k chunks; the
                                          ones column makes row 64 = sum_k P
                                          = softmax denominator, for free)
  out^T = rawout^T[:64] * (1/denom)      (DVE recip + PE K=1 broadcast + DVE)
  y^T(partial) = Wp_g^T.T @ out^T        (PE)
"""

import os
import sys
import types

for _p in ("/opt/trn_rl_repo", "/root/.axon_site", "/root/.axon_site/_ro/trn_rl_repo"):
    if os.path.isdir(_p) and _p not in sys.path:
        sys.path.append(_p)

import numpy as np

import concourse.bacc as bacc
import concourse.bass as bass
import concourse.mybir as mybir
import concourse.tile as tile
from concourse.bass_utils import run_bass_kernel_spmd

# ── problem constants (hardcoded; spec.json not available at grade time) ──
B, T, C = 2, 2048, 1024
H, D = 16, 64
N_CORES = 8
HPG = 4                 # heads per group (per core)
CG = HPG * D            # 256 channels per head-group
NT = T // 512           # 4 token chunks of 512
KC = C // 128           # 8 contraction tiles for C
NEG = -1.0e4            # (unused) additive mask value

F32 = mybir.dt.float32
F32R = mybir.dt.float32r
# per-stage matmul operand dtype: float32 (exact, 4 cyc/row) or float32r
# (tf32-like, 1 cyc/row at N>=256). Overridable for A/B testing.
MMDT = {
    "qk": F32R, "v": F32R, "att": F32R, "av": F32R, "proj": F32R, "k1": F32,
}
_trace_flag = [False]   # test.py can flip this to capture a profile
_last_results = [None]


def _mm(nc, out, lhsT, rhs, stage, **kw):
    nc.tensor.matmul(out, lhsT, rhs, **kw)


def _ensure_ntff_hook():
    """Install the NTFF profile hook shim (container's antenv lacks it)."""
    if "antenv.axon_hooks" in sys.modules:
        return
    try:
        from trn_agent_boot.trn_boot import _ntff_profile_via_ctypes
    except Exception:
        return
    mod = types.ModuleType("antenv.axon_hooks")
    hook = [None]
    mod.set_axon_ntff_profile_hook = lambda h: hook.__setitem__(0, h)
    mod.get_axon_ntff_profile_hook = lambda: hook[0]
    sys.modules["antenv.axon_hooks"] = mod
    so = "/opt/axon/libaxon_pjrt.so"
    if os.path.exists(so):
        mod.set_axon_ntff_profile_hook(_ntff_profile_via_ctypes(so))


def build_nc():
    nc = bacc.Bacc("TRN2", target_bir_lowering=False, debug=False,
                   num_devices=N_CORES)

    xt_d = nc.dram_tensor("xt", [C, T], F32, kind="ExternalInput").ap()
    wqk_d = nc.dram_tensor("wqk", [C, 2 * CG], F32, kind="ExternalInput").ap()
    bqk_d = nc.dram_tensor("bqk", [2 * CG, 1], F32, kind="ExternalInput").ap()
    wv_d = nc.dram_tensor("wv", [C, HPG * 65], F32, kind="ExternalInput").ap()
    bv_d = nc.dram_tensor("bv", [1, HPG * 65], F32, kind="ExternalInput").ap()
    wp_d = nc.dram_tensor("wp", [CG, C], F32, kind="ExternalInput").ap()
    mask_d = nc.dram_tensor("mask", [128, 128], F32, kind="ExternalInput").ap()
    ones_d = nc.dram_tensor("ones", [1, 128], F32, kind="ExternalInput").ap()
    yt_d = nc.dram_tensor("yt", [C, T], F32, kind="ExternalOutput").ap()
    rec_d = nc.dram_tensor("rec_scratch", [HPG * NT, 512], F32).ap()
    den_d = nc.dram_tensor("den_scratch", [HPG * NT, 512], F32).ap()

    with tile.TileContext(nc) as tc:
        with tc.tile_pool(name="const", bufs=1) as cp:
            # ── persistent SBUF residents ──
            assert MMDT["v"] == MMDT["qk"]
            xtp = tc.tile_pool(name="xtp", bufs=1)
            xtpool = xtp.__enter__()
            xt = [xtpool.tile([128, T], MMDT["qk"], tag=f"xt{k}", name=f"xt{k}") for k in range(KC)]
            wqk = [cp.tile([128, 2 * CG], MMDT["qk"], tag=f"wqk{k}", name=f"wqk{k}") for k in range(KC)]
            wv = [cp.tile([128, HPG * 65], MMDT["v"], tag=f"wv{k}", name=f"wv{k}") for k in range(KC)]
            bqk = [cp.tile([128, 1], F32, tag=f"bqk{m}", name=f"bqk{m}") for m in range(4)]
            bv = cp.tile([1, HPG * 65], MMDT["v"], tag="bv")
            wp = [cp.tile([128, C], MMDT["proj"], tag=f"wp{k}", name=f"wp{k}") for k in range(2)]
            tri = cp.tile([128, 128], MMDT["av"], tag="tri", name="tri")
            ones = cp.tile([1, 128], MMDT["v"], tag="ones")
            qk = [cp.tile([128, T], MMDT["att"], tag=f"qk{m}", name=f"qk{m}") for m in range(4)]
            v_sb = [cp.tile([128, HPG * 65], MMDT["av"], tag=f"v{m}", name=f"v{m}") for m in range(T // 128)]
            outT = [cp.tile([128, T], MMDT["proj"], tag=f"outT{k}", name=f"outT{k}") for k in range(2)]

            for k in range(KC):
                nc.sync.dma_start(xt[k][:], xt_d[128 * k:128 * (k + 1), :].bitcast(MMDT['qk']))
                nc.sync.dma_start(wqk[k][:], wqk_d[128 * k:128 * (k + 1), :].bitcast(MMDT['qk']))
                nc.sync.dma_start(wv[k][:], wv_d[128 * k:128 * (k + 1), :].bitcast(MMDT['v']))
            for m in range(4):
                nc.sync.dma_start(bqk[m][:], bqk_d[128 * m:128 * (m + 1), :])
            nc.sync.dma_start(bv[:], bv_d[:].bitcast(MMDT['v']))
            for k in range(2):
                nc.sync.dma_start(wp[k][:], wp_d[128 * k:128 * (k + 1), :].bitcast(MMDT['proj']))
            nc.sync.dma_start(tri[:], mask_d[:].bitcast(MMDT['av']))
            nc.sync.dma_start(ones[:], ones_d[:].bitcast(MMDT['v']))

            # ── stage B: qk^T [512, T] = wqk.T @ xt, bias on evacuation ──
            with tc.tile_pool(name="psB", bufs=3, space="PSUM") as psB:
                for mf in range(4):
                    for nt in range(NT):
                        ps = psB.tile([128, 512], F32)
                        for k in range(KC):
                            _mm(nc, ps[:], wqk[k][:, 128 * mf:128 * (mf + 1)],
                                xt[k][:, 512 * nt:512 * (nt + 1)], "qk",
                                start=(k == 0), stop=(k == KC - 1))
                        nc.vector.tensor_scalar_add(
                            qk[mf][:, 512 * nt:512 * (nt + 1)], ps[:],
                            bqk[mf][:])

                # ── stage C: v_aug [T, 260] = xt.T @ wv (+ ones col via K=1) ──
                for mt in range(T // 128):
                    ps = psB.tile([128, HPG * 65], F32, tag="psv")
                    for k in range(KC):
                        _mm(nc, ps[:], xt[k][:, 128 * mt:128 * (mt + 1)],
                            wv[k][:], "v", start=(k == 0), stop=False)
                    _mm(nc, ps[:], ones[:, :], bv[:], "k1", start=False,
                        stop=True)
                    nc.vector.tensor_copy(v_sb[mt][:], ps[:])


            # ── stage D: attention. All matmuls keep base partition 0 —
            # alternating base partitions between attT (64-row) and av
            # (128-row) matmuls measured ~1.5x slower on HW. Odd heads'
            # q/k rows live at partitions 64-127, so DMA-shift them down
            # to a base-0 scratch tile first. ──
            LAG = 2   # av lags attT by LAG tiles to hide the exp latency
            with (
                tc.tile_pool(name="psA", bufs=3, space="PSUM") as psA,
                tc.tile_pool(name="psAV", bufs=3, space="PSUM") as psAV,
                tc.tile_pool(name="expp", bufs=4) as expp,
                tc.tile_pool(name="recp", bufs=2) as recp,
                tc.tile_pool(name="rawp", bufs=2) as rawp,
                tc.tile_pool(name="bcp", bufs=2) as bcp,
                tc.tile_pool(name="shp", bufs=1) as shp,
            ):
                shifted = {}
                def shift_head(h):
                    # copy odd head h's q/k rows (partitions 64-127) down to 0-63
                    qtile, ktile = h // 2, 2 + h // 2
                    qs = shp.tile([64, T], MMDT["att"], tag="qs", name=f"qs{h}")
                    ks = shp.tile([64, T], MMDT["att"], tag="ks", name=f"ks{h}")
                    nc.sync.dma_start(qs[:], qk[qtile][64:128, :])
                    nc.sync.dma_start(ks[:], qk[ktile][64:128, :])
                    shifted[h] = (qs, ks)
                shift_head(1)
                for h in range(HPG):
                    if h == 1:
                        shift_head(3)
                    if h % 2 == 0:
                        qt_ap, kt_ap = qk[h // 2][0:64, :], qk[2 + h // 2][0:64, :]
                    else:
                        qt_ap, kt_ap = shifted.pop(h)
                        qt_ap, kt_ap = qt_ap[:, :], kt_ap[:, :]
                    for j in range(NT):
                        u = h * NT + j
                        n_i = 4 * (j + 1)        # causal: k-chunks 0 .. 4j+3
                        avps = psAV.tile([65, 512], F32, tag="avps",
                                         name=f"avps{h}_{j}")
                        ets = {}
                        for i in range(n_i):
                            p = i - 4 * j        # >=0 on diagonal tiles
                            c0 = 128 * p if p > 0 else 0
                            aps = psA.tile([128, 512], F32, tag="aps",
                                           name=f"aps{h}_{j}_{i}")
                            _mm(nc, aps[:, c0:512],
                                kt_ap[:, 128 * i:128 * (i + 1)],
                                qt_ap[:, 512 * j + c0:512 * (j + 1)],
                                "att", start=True, stop=True)
                            et = expp.tile([128, 512], MMDT["av"], tag="et",
                                           name=f"et{h}_{j}_{i}")
                            nc.scalar.activation(et[:, c0:512], aps[:, c0:512],
                                                 mybir.ActivationFunctionType.Exp)
                            if p >= 0:  # triangular block at cols [c0, c0+128)
                                nc.vector.tensor_mul(et[:, c0:c0 + 128],
                                                     et[:, c0:c0 + 128],
                                                     tri[:])
                            ets[i] = et
                            if i >= LAG:
                                ii = i - LAG
                                cc = max(0, 128 * (ii - 4 * j))
                                _mm(nc, avps[:, cc:512],
                                    v_sb[ii][:, 65 * h:65 * h + 65],
                                    ets.pop(ii)[:, cc:512], "av",
                                    start=(ii == 0), stop=False)
                        for ii in sorted(ets):
                            cc = max(0, 128 * (ii - 4 * j))
                            _mm(nc, avps[:, cc:512],
                                v_sb[ii][:, 65 * h:65 * h + 65],
                                ets.pop(ii)[:, cc:512], "av",
                                start=(ii == 0), stop=(ii == n_i - 1))
                        # evacuate rawout+denominator, free the PSUM bank
                        raw = rawp.tile([65, 512], F32, tag="raw",
                                        name=f"raw{h}_{j}")
                        nc.vector.tensor_copy(raw[:], avps[:])
                        nc.sync.dma_start(den_d[u:u + 1, :], raw[64:65, :])
                        # reciprocal remapped to [128, 4]: all lanes share work
                        den2 = recp.tile([128, 4], F32, tag="den2",
                                         name=f"den2_{h}_{j}")
                        nc.sync.dma_start(
                            den2[:], bass.AP(den_d.tensor, u * 512,
                                             [[4, 128], [1, 4]]))
                        rec2 = recp.tile([128, 4], F32, tag="rec2",
                                         name=f"rec2_{h}_{j}")
                        nc.vector.reciprocal(rec2[:], den2[:])
                        nc.sync.dma_start(
                            bass.AP(rec_d.tensor, u * 512, [[4, 128], [1, 4]]),
                            rec2[:])
                        bc_sb = bcp.tile([64, 512], F32, tag="bc",
                                         name=f"bc{h}_{j}")
                        nc.sync.dma_start(
                            bc_sb[:], bass.AP(rec_d.tensor, u * 512,
                                              [[0, 64], [1, 512]]))
                        off = 64 * (h % 2)
                        nc.vector.tensor_mul(
                            outT[h // 2][off:off + 64, 512 * j:512 * (j + 1)],
                            raw[0:64, :], bc_sb[:])

            # ── stage E: y^T partial [C, T] = wp.T @ outT ──
            with (
                tc.tile_pool(name="psP", bufs=3, space="PSUM") as psP,
                tc.tile_pool(name="outp", bufs=3) as outp,
            ):
                for mo in range(8):
                    for nt in range(NT):
                        ps = psP.tile([128, 512], F32)
                        for k in range(2):
                            _mm(nc, ps[:], wp[k][:, 128 * mo:128 * (mo + 1)],
                                outT[k][:, 512 * nt:512 * (nt + 1)], "proj",
                                start=(k == 0), stop=(k == 1))
                        ot = outp.tile([128, 512], F32)
                        nc.vector.tensor_copy(ot[:], ps[:])
                        nc.sync.dma_start(
                            yt_d[128 * mo:128 * (mo + 1),
                                 512 * nt:512 * (nt + 1)], ot[:])

            xtp.__exit__(None, None, None)

    nc.compile()
    return nc


def _shard_inputs(x, w_qkv, b_qkv, w_proj):
    scale = 1.0 / np.sqrt(D)   # 0.125, exact power of two
    in_maps = []
    r = np.arange(128)[:, None]
    c = np.arange(128)[None, :]
    mask = np.where(c >= r, 1.0, 0.0).astype(np.float32)
    for core in range(N_CORES):
        b, g = divmod(core, HPG)
        qs = slice(CG * g, CG * (g + 1))
        ks = slice(C + CG * g, C + CG * (g + 1))
        vs = slice(2 * C + CG * g, 2 * C + CG * (g + 1))
        wqk = np.concatenate([w_qkv[qs] * scale, w_qkv[ks]], axis=0).T
        bqk = np.concatenate([b_qkv[qs] * scale, b_qkv[ks]])[:, None]
        wv_base = w_qkv[vs].T          # [C, 256]
        wv = np.zeros((C, HPG * 65), np.float32)
        bv = np.zeros((1, HPG * 65), np.float32)
        for h in range(HPG):
            wv[:, 65 * h:65 * h + 64] = wv_base[:, 64 * h:64 * h + 64]
            bv[0, 65 * h:65 * h + 64] = b_qkv[vs][64 * h:64 * h + 64]
            bv[0, 65 * h + 64] = 1.0
        in_maps.append({
            "xt": np.ascontiguousarray(x[b].T, np.float32),
            "wqk": np.ascontiguousarray(wqk, np.float32),
            "bqk": np.ascontiguousarray(bqk, np.float32),
            "wv": wv,
            "bv": bv,
            "wp": np.ascontiguousarray(w_proj[:, CG * g:CG * (g + 1)].T,
                                       np.float32),
            "mask": mask,
            "ones": np.ones((1, 128), np.float32),
        })
    return in_maps


def kernel(x, w_qkv, b_qkv, w_proj, b_proj):
    x = np.asarray(x, np.float32)
    w_qkv = np.asarray(w_qkv, np.float32)
    b_qkv = np.asarray(b_qkv, np.float32)
    w_proj = np.asarray(w_proj, np.float32)
    b_proj = np.asarray(b_proj, np.float32)

    nc = build_nc()
    in_maps = _shard_inputs(x, w_qkv, b_qkv, w_proj)
    if _trace_flag[0]:
        _ensure_ntff_hook()
    res = run_bass_kernel_spmd(nc, in_maps, core_ids=list(range(N_CORES)),
                               trace=_trace_flag[0])
    _last_results[0] = res

    y = np.empty((B, T, C), np.float32)
    for b in range(B):
        acc = np.zeros((C, T), np.float32)
        for g in range(HPG):
            acc += res.results[HPG * b + g]["yt"]
        y[b] = acc.T + b_proj[None, :]
    return y


# revision 21
# speedup vs baseline: 1.0060x; 1.0060x over previous
"""Causal self-attention (B=2, T=2048, C=1024, H=16, D=64) on 8 trn2 cores.

Sharding: core c -> batch b = c // 4, head-group g = c % 4 (4 heads each).
Data-parallel over B, tensor-parallel (Megatron) over heads for the
qkv / proj linears. Each core computes its head-group's attention and a
partial output projection; the host sums the 4 partials per batch and
adds the proj bias.

Everything on-device is done in transposed [feature, token] space:
  qk^T = Wqk @ x^T                       (PE; bias added by ACT on evacuate)
  v    = x @ Wv^T (+ ones column)        (PE; K=1 matmul adds bias + ones)
  att^T[k, q] = k^T(head)^T . q(head)    (PE, K=64; causal tiles skipped)
  P = exp(att^T + additive causal mask)  (ACT; no max-subtraction needed,
                                          |logits| <~ 10 so fp32 exp is safe)
  rawout^T[d+1, q] = v_aug^T @ P         (PE accumulate over k chunks; the
                                          ones column makes row 64 = sum_k P
                                          = softmax denominator, for free)
  out^T = rawout^T[:64] * (1/denom)      (DVE recip + PE K=1 broadcast + DVE)
  y^T(partial) = Wp_g^T.T @ out^T        (PE)
"""

import os
import sys
import types

for _p in ("/opt/trn_rl_repo", "/root/.axon_site", "/root/.axon_site/_ro/trn_rl_repo"):
    if os.path.isdir(_p) and _p not in sys.path:
        sys.path.append(_p)

import numpy as np

import concourse.bacc as bacc
import concourse.bass as bass
import concourse.mybir as mybir
import concourse.tile as tile
from concourse.bass_utils import run_bass_kernel_spmd

# ── problem constants (hardcoded; spec.json not available at grade time) ──
B, T, C = 2, 2048, 1024
H, D = 16, 64
N_CORES = 8
HPG = 4                 # heads per group (per core)
CG = HPG * D            # 256 channels per head-group
NT = T // 512           # 4 token chunks of 512
KC = C // 128           # 8 contraction tiles for C
NEG = -1.0e4            # (unused) additive mask value

F32 = mybir.dt.float32
F32R = mybir.dt.float32r
# per-stage matmul operand dtype: float32 (exact, 4 cyc/row) or float32r
# (tf32-like, 1 cyc/row at N>=256). Overridable for A/B testing.
MMDT = {
    "qk": F32R, "v": F32R, "att": F32R, "av": F32R, "proj": F32R, "k1": F32,
}
_trace_flag = [False]   # test.py can flip this to capture a profile
_last_results = [None]


def _mm(nc, out, lhsT, rhs, stage, **kw):
    nc.tensor.matmul(out, lhsT, rhs, **kw)


def _ensure_ntff_hook():
    """Install the NTFF profile hook shim (container's antenv lacks it)."""
    if "antenv.axon_hooks" in sys.modules:
        return
    try:
        from trn_agent_boot.trn_boot import _ntff_profile_via_ctypes
    except Exception:
        return
    mod = types.ModuleType("antenv.axon_hooks")
    hook = [None]
    mod.set_axon_ntff_profile_hook = lambda h: hook.__setitem__(0, h)
    mod.get_axon_ntff_profile_hook = lambda: hook[0]
    sys.modules["antenv.axon_hooks"] = mod
    so = "/opt/axon/libaxon_pjrt.so"
    if os.path.exists(so):
        mod.set_axon_ntff_profile_hook(_ntff_profile_via_ctypes(so))


def build_nc():
    nc = bacc.Bacc("TRN2", target_bir_lowering=False, debug=False,
                   num_devices=N_CORES)

    xt_d = nc.dram_tensor("xt", [C, T], F32, kind="ExternalInput").ap()
    wqk_d = nc.dram_tensor("wqk", [C, 2 * CG], F32, kind="ExternalInput").ap()
    bqk_d = nc.dram_tensor("bqk", [2 * CG, 1], F32, kind="ExternalInput").ap()
    wv_d = nc.dram_tensor("wv", [C, HPG * 65], F32, kind="ExternalInput").ap()
    bv_d = nc.dram_tensor("bv", [1, HPG * 65], F32, kind="ExternalInput").ap()
    wp_d = nc.dram_tensor("wp", [CG, C], F32, kind="ExternalInput").ap()
    mask_d = nc.dram_tensor("mask", [128, 128], F32, kind="ExternalInput").ap()
    ones_d = nc.dram_tensor("ones", [1, 128], F32, kind="ExternalInput").ap()
    yt_d = nc.dram_tensor("yt", [C, T], F32, kind="ExternalOutput").ap()
    rec_d = nc.dram_tensor("rec_scratch", [HPG * NT, 512], F32).ap()
    den_d = nc.dram_tensor("den_scratch", [HPG * NT, 512], F32).ap()

    with tile.TileContext(nc) as tc:
        with tc.tile_pool(name="const", bufs=1) as cp:
            # ── persistent SBUF residents ──
            assert MMDT["v"] == MMDT["qk"]
            xtp = tc.tile_pool(name="xtp", bufs=1)
            xtpool = xtp.__enter__()
            xt = [xtpool.tile([128, T], MMDT["qk"], tag=f"xt{k}", name=f"xt{k}") for k in range(KC)]
            wqk = [cp.tile([128, 2 * CG], MMDT["qk"], tag=f"wqk{k}", name=f"wqk{k}") for k in range(KC)]
            wv = [cp.tile([128, HPG * 65], MMDT["v"], tag=f"wv{k}", name=f"wv{k}") for k in range(KC)]
            bqk = [cp.tile([128, 1], F32, tag=f"bqk{m}", name=f"bqk{m}") for m in range(4)]
            bv = cp.tile([1, HPG * 65], MMDT["v"], tag="bv")
            wp = [cp.tile([128, C], MMDT["proj"], tag=f"wp{k}", name=f"wp{k}") for k in range(2)]
            tri = cp.tile([128, 128], MMDT["av"], tag="tri", name="tri")
            ones = cp.tile([1, 128], MMDT["v"], tag="ones")
            qk = [cp.tile([128, T], MMDT["att"], tag=f"qk{m}", name=f"qk{m}") for m in range(4)]
            v_sb = [cp.tile([128, HPG * 65], MMDT["av"], tag=f"v{m}", name=f"v{m}") for m in range(T // 128)]
            outT = [cp.tile([128, T], MMDT["proj"], tag=f"outT{k}", name=f"outT{k}") for k in range(2)]

            for k in range(KC):
                nc.sync.dma_start(xt[k][:], xt_d[128 * k:128 * (k + 1), :].bitcast(MMDT['qk']))
                nc.sync.dma_start(wqk[k][:], wqk_d[128 * k:128 * (k + 1), :].bitcast(MMDT['qk']))
                nc.sync.dma_start(wv[k][:], wv_d[128 * k:128 * (k + 1), :].bitcast(MMDT['v']))
            for m in range(4):
                nc.sync.dma_start(bqk[m][:], bqk_d[128 * m:128 * (m + 1), :])
            nc.sync.dma_start(bv[:], bv_d[:].bitcast(MMDT['v']))
            for k in range(2):
                nc.sync.dma_start(wp[k][:], wp_d[128 * k:128 * (k + 1), :].bitcast(MMDT['proj']))
            nc.sync.dma_start(tri[:], mask_d[:].bitcast(MMDT['av']))
            nc.sync.dma_start(ones[:], ones_d[:].bitcast(MMDT['v']))

            # ── stage B: qk^T [512, T] = wqk.T @ xt, bias on evacuation ──
            with tc.tile_pool(name="psB", bufs=3, space="PSUM") as psB:
                for mf in range(4):
                    for nt in range(NT):
                        ps = psB.tile([128, 512], F32)
                        for k in range(KC):
                            _mm(nc, ps[:], wqk[k][:, 128 * mf:128 * (mf + 1)],
                                xt[k][:, 512 * nt:512 * (nt + 1)], "qk",
                                start=(k == 0), stop=(k == KC - 1))
                        nc.vector.tensor_scalar_add(
                            qk[mf][:, 512 * nt:512 * (nt + 1)], ps[:],
                            bqk[mf][:])

                # ── stage C: v_aug [T, 260] = xt.T @ wv (+ ones col via K=1) ──
                for mt in range(T // 128):
                    ps = psB.tile([128, HPG * 65], F32, tag="psv")
                    for k in range(KC):
                        _mm(nc, ps[:], xt[k][:, 128 * mt:128 * (mt + 1)],
                            wv[k][:], "v", start=(k == 0), stop=False)
                    _mm(nc, ps[:], ones[:, :], bv[:], "k1", start=False,
                        stop=True)
                    nc.vector.tensor_copy(v_sb[mt][:], ps[:])


            # ── stage D: attention. All matmuls keep base partition 0 —
            # alternating base partitions between attT (64-row) and av
            # (128-row) matmuls measured ~1.5x slower on HW. Odd heads'
            # q/k rows live at partitions 64-127, so DMA-shift them down
            # to a base-0 scratch tile first. ──
            LAG = 4   # av lags attT by LAG tiles to hide the exp latency
            with (
                tc.tile_pool(name="psA", bufs=4, space="PSUM") as psA,
                tc.tile_pool(name="psAV", bufs=3, space="PSUM") as psAV,
                tc.tile_pool(name="expp", bufs=6) as expp,
                tc.tile_pool(name="recp", bufs=2) as recp,
                tc.tile_pool(name="rawp", bufs=2) as rawp,
                tc.tile_pool(name="bcp", bufs=2) as bcp,
                tc.tile_pool(name="shp", bufs=1) as shp,
            ):
                shifted = {}
                def shift_head(h):
                    # copy odd head h's q/k rows (partitions 64-127) down to 0-63
                    qtile, ktile = h // 2, 2 + h // 2
                    qs = shp.tile([64, T], MMDT["att"], tag="qs", name=f"qs{h}")
                    ks = shp.tile([64, T], MMDT["att"], tag="ks", name=f"ks{h}")
                    nc.sync.dma_start(qs[:], qk[qtile][64:128, :])
                    nc.sync.dma_start(ks[:], qk[ktile][64:128, :])
                    shifted[h] = (qs, ks)
                shift_head(1)
                for h in range(HPG):
                    if h == 1:
                        shift_head(3)
                    if h % 2 == 0:
                        qt_ap, kt_ap = qk[h // 2][0:64, :], qk[2 + h // 2][0:64, :]
                    else:
                        qt_ap, kt_ap = shifted.pop(h)
                        qt_ap, kt_ap = qt_ap[:, :], kt_ap[:, :]
                    for j in range(NT):
                        u = h * NT + j
                        n_i = 4 * (j + 1)        # causal: k-chunks 0 .. 4j+3
                        avps = psAV.tile([65, 512], F32, tag="avps",
                                         name=f"avps{h}_{j}")
                        ets = {}
                        for i in range(n_i):
                            p = i - 4 * j        # >=0 on diagonal tiles
                            c0 = 128 * p if p > 0 else 0
                            aps = psA.tile([128, 512], F32, tag="aps",
                                           name=f"aps{h}_{j}_{i}")
                            _mm(nc, aps[:, c0:512],
                                kt_ap[:, 128 * i:128 * (i + 1)],
                                qt_ap[:, 512 * j + c0:512 * (j + 1)],
                                "att", start=True, stop=True)
                            et = expp.tile([128, 512], MMDT["av"], tag="et",
                                           name=f"et{h}_{j}_{i}")
                            nc.scalar.activation(et[:, c0:512], aps[:, c0:512],
                                                 mybir.ActivationFunctionType.Exp)
                            if p >= 0:  # triangular block at cols [c0, c0+128)
                                nc.vector.tensor_mul(et[:, c0:c0 + 128],
                                                     et[:, c0:c0 + 128],
                                                     tri[:])
                            ets[i] = et
                            if i >= LAG:
                                ii = i - LAG
                                cc = max(0, 128 * (ii - 4 * j))
                                _mm(nc, avps[:, cc:512],
                                    v_sb[ii][:, 65 * h:65 * h + 65],
                                    ets.pop(ii)[:, cc:512], "av",
                                    start=(ii == 0), stop=False)
                        for ii in sorted(ets):
                            cc = max(0, 128 * (ii - 4 * j))
                            _mm(nc, avps[:, cc:512],
                                v_sb[ii][:, 65 * h:65 * h + 65],
                                ets.pop(ii)[:, cc:512], "av",
                                start=(ii == 0), stop=(ii == n_i - 1))
                        # evacuate rawout+denominator, free the PSUM bank
                        raw = rawp.tile([65, 512], F32, tag="raw",
                                        name=f"raw{h}_{j}")
                        nc.vector.tensor_copy(raw[:], avps[:])
                        nc.sync.dma_start(den_d[u:u + 1, :], raw[64:65, :])
                        # reciprocal remapped to [128, 4]: all lanes share work
                        den2 = recp.tile([128, 4], F32, tag="den2",
                                         name=f"den2_{h}_{j}")
                        nc.sync.dma_start(
                            den2[:], bass.AP(den_d.tensor, u * 512,
                                             [[4, 128], [1, 4]]))
                        rec2 = recp.tile([128, 4], F32, tag="rec2",
                                         name=f"rec2_{h}_{j}")
                        nc.vector.reciprocal(rec2[:], den2[:])
                        nc.sync.dma_start(
                            bass.AP(rec_d.tensor, u * 512, [[4, 128], [1, 4]]),
                            rec2[:])
                        bc_sb = bcp.tile([64, 512], F32, tag="bc",
                                         name=f"bc{h}_{j}")
                        nc.sync.dma_start(
                            bc_sb[:], bass.AP(rec_d.tensor, u * 512,
                                              [[0, 64], [1, 512]]))
                        off = 64 * (h % 2)
                        nc.vector.tensor_mul(
                            outT[h // 2][off:off + 64, 512 * j:512 * (j + 1)],
                            raw[0:64, :], bc_sb[:])

            # ── stage E: y^T partial [C, T] = wp.T @ outT ──
            with (
                tc.tile_pool(name="psP", bufs=3, space="PSUM") as psP,
                tc.tile_pool(name="outp", bufs=3) as outp,
            ):
                for mo in range(8):
                    for nt in range(NT):
                        ps = psP.tile([128, 512], F32)
                        for k in range(2):
                            _mm(nc, ps[:], wp[k][:, 128 * mo:128 * (mo + 1)],
                                outT[k][:, 512 * nt:512 * (nt + 1)], "proj",
                                start=(k == 0), stop=(k == 1))
                        ot = outp.tile([128, 512], F32)
                        nc.vector.tensor_copy(ot[:], ps[:])
                        nc.sync.dma_start(
                            yt_d[128 * mo:128 * (mo + 1),
                                 512 * nt:512 * (nt + 1)], ot[:])

            xtp.__exit__(None, None, None)

    nc.compile()
    return nc


def _shard_inputs(x, w_qkv, b_qkv, w_proj):
    scale = 1.0 / np.sqrt(D)   # 0.125, exact power of two
    in_maps = []
    r = np.arange(128)[:, None]
    c = np.arange(128)[None, :]
    mask = np.where(c >= r, 1.0, 0.0).astype(np.float32)
    for core in range(N_CORES):
        b, g = divmod(core, HPG)
        qs = slice(CG * g, CG * (g + 1))
        ks = slice(C + CG * g, C + CG * (g + 1))
        vs = slice(2 * C + CG * g, 2 * C + CG * (g + 1))
        wqk = np.concatenate([w_qkv[qs] * scale, w_qkv[ks]], axis=0).T
        bqk = np.concatenate([b_qkv[qs] * scale, b_qkv[ks]])[:, None]
        wv_base = w_qkv[vs].T          # [C, 256]
        wv = np.zeros((C, HPG * 65), np.float32)
        bv = np.zeros((1, HPG * 65), np.float32)
        for h in range(HPG):
            wv[:, 65 * h:65 * h + 64] = wv_base[:, 64 * h:64 * h + 64]
            bv[0, 65 * h:65 * h + 64] = b_qkv[vs][64 * h:64 * h + 64]
            bv[0, 65 * h + 64] = 1.0
        in_maps.append({
            "xt": np.ascontiguousarray(x[b].T, np.float32),
            "wqk": np.ascontiguousarray(wqk, np.float32),
            "bqk": np.ascontiguousarray(bqk, np.float32),
            "wv": wv,
            "bv": bv,
            "wp": np.ascontiguousarray(w_proj[:, CG * g:CG * (g + 1)].T,
                                       np.float32),
            "mask": mask,
            "ones": np.ones((1, 128), np.float32),
        })
    return in_maps


def kernel(x, w_qkv, b_qkv, w_proj, b_proj):
    x = np.asarray(x, np.float32)
    w_qkv = np.asarray(w_qkv, np.float32)
    b_qkv = np.asarray(b_qkv, np.float32)
    w_proj = np.asarray(w_proj, np.float32)
    b_proj = np.asarray(b_proj, np.float32)

    nc = build_nc()
    in_maps = _shard_inputs(x, w_qkv, b_qkv, w_proj)
    if _trace_flag[0]:
        _ensure_ntff_hook()
    res = run_bass_kernel_spmd(nc, in_maps, core_ids=list(range(N_CORES)),
                               trace=_trace_flag[0])
    _last_results[0] = res

    y = np.empty((B, T, C), np.float32)
    for b in range(B):
        acc = np.zeros((C, T), np.float32)
        for g in range(HPG):
            acc += res.results[HPG * b + g]["yt"]
        y[b] = acc.T + b_proj[None, :]
    return y


# revision 23
# speedup vs baseline: 1.0307x; 1.0245x over previous
"""Causal self-attention (B=2, T=2048, C=1024, H=16, D=64) on 8 trn2 cores.

Sharding: core c -> batch b = c // 4, head-group g = c % 4 (4 heads each).
Data-parallel over B, tensor-parallel (Megatron) over heads for the
qkv / proj linears. Each core computes its head-group's attention and a
partial output projection; the host sums the 4 partials per batch and
adds the proj bias.

Everything on-device is done in transposed [feature, token] space:
  qk^T = Wqk @ x^T                       (PE; bias added by ACT on evacuate)
  v    = x @ Wv^T (+ ones column)        (PE; K=1 matmul adds bias + ones)
  att^T[k, q] = k^T(head)^T . q(head)    (PE, K=64; causal tiles skipped)
  P = exp(att^T + additive causal mask)  (ACT; no max-subtraction needed,
                                          |logits| <~ 10 so fp32 exp is safe)
  rawout^T[d+1, q] = v_aug^T @ P         (PE accumulate over k chunks; the
                                          ones column makes row 64 = sum_k P
                                          = softmax denominator, for free)
  out^T = rawout^T[:64] * (1/denom)      (DVE recip + PE K=1 broadcast + DVE)
  y^T(partial) = Wp_g^T.T @ out^T        (PE)
"""

import os
import sys
import types

for _p in ("/opt/trn_rl_repo", "/root/.axon_site", "/root/.axon_site/_ro/trn_rl_repo"):
    if os.path.isdir(_p) and _p not in sys.path:
        sys.path.append(_p)

import numpy as np

import concourse.bacc as bacc
import concourse.bass as bass
import concourse.mybir as mybir
import concourse.tile as tile
from concourse.bass_utils import run_bass_kernel_spmd

# ── problem constants (hardcoded; spec.json not available at grade time) ──
B, T, C = 2, 2048, 1024
H, D = 16, 64
N_CORES = 8
HPG = 4                 # heads per group (per core)
CG = HPG * D            # 256 channels per head-group
NT = T // 512           # 4 token chunks of 512
KC = C // 128           # 8 contraction tiles for C
NEG = -1.0e4            # (unused) additive mask value

F32 = mybir.dt.float32
F32R = mybir.dt.float32r
# per-stage matmul operand dtype: float32 (exact, 4 cyc/row) or float32r
# (tf32-like, 1 cyc/row at N>=256). Overridable for A/B testing.
MMDT = {
    "qk": F32R, "v": F32R, "att": F32R, "av": F32R, "proj": F32R, "k1": F32,
}
_trace_flag = [False]   # test.py can flip this to capture a profile
_last_results = [None]


def _mm(nc, out, lhsT, rhs, stage, **kw):
    nc.tensor.matmul(out, lhsT, rhs, **kw)


def _ensure_ntff_hook():
    """Install the NTFF profile hook shim (container's antenv lacks it)."""
    if "antenv.axon_hooks" in sys.modules:
        return
    try:
        from trn_agent_boot.trn_boot import _ntff_profile_via_ctypes
    except Exception:
        return
    mod = types.ModuleType("antenv.axon_hooks")
    hook = [None]
    mod.set_axon_ntff_profile_hook = lambda h: hook.__setitem__(0, h)
    mod.get_axon_ntff_profile_hook = lambda: hook[0]
    sys.modules["antenv.axon_hooks"] = mod
    so = "/opt/axon/libaxon_pjrt.so"
    if os.path.exists(so):
        mod.set_axon_ntff_profile_hook(_ntff_profile_via_ctypes(so))


def build_nc():
    nc = bacc.Bacc("TRN2", target_bir_lowering=False, debug=False,
                   num_devices=N_CORES)

    xt_d = nc.dram_tensor("xt", [C, T], F32, kind="ExternalInput").ap()
    wqk_d = nc.dram_tensor("wqk", [C, 2 * CG], F32, kind="ExternalInput").ap()
    bqk_d = nc.dram_tensor("bqk", [2 * CG, 1], F32, kind="ExternalInput").ap()
    wv_d = nc.dram_tensor("wv", [C, HPG * 65], F32, kind="ExternalInput").ap()
    bv_d = nc.dram_tensor("bv", [1, HPG * 65], F32, kind="ExternalInput").ap()
    wp_d = nc.dram_tensor("wp", [CG, C], F32, kind="ExternalInput").ap()
    mask_d = nc.dram_tensor("mask", [128, 128], F32, kind="ExternalInput").ap()
    ones_d = nc.dram_tensor("ones", [1, 128], F32, kind="ExternalInput").ap()
    yt_d = nc.dram_tensor("yt", [C, T], F32, kind="ExternalOutput").ap()
    rec_d = nc.dram_tensor("rec_scratch", [HPG * NT, 512], F32).ap()
    den_d = nc.dram_tensor("den_scratch", [HPG * NT, 512], F32).ap()

    with tile.TileContext(nc) as tc:
        with tc.tile_pool(name="const", bufs=1) as cp:
            # ── persistent SBUF residents ──
            assert MMDT["v"] == MMDT["qk"]
            xtp = tc.tile_pool(name="xtp", bufs=1)
            xtpool = xtp.__enter__()
            xt = [xtpool.tile([128, T], MMDT["qk"], tag=f"xt{k}", name=f"xt{k}") for k in range(KC)]
            wqk = [cp.tile([128, 2 * CG], MMDT["qk"], tag=f"wqk{k}", name=f"wqk{k}") for k in range(KC)]
            wv = [cp.tile([128, HPG * 65], MMDT["v"], tag=f"wv{k}", name=f"wv{k}") for k in range(KC)]
            bqk = [cp.tile([128, 1], F32, tag=f"bqk{m}", name=f"bqk{m}") for m in range(4)]
            bv = cp.tile([1, HPG * 65], MMDT["v"], tag="bv")
            wp = [cp.tile([128, C], MMDT["proj"], tag=f"wp{k}", name=f"wp{k}") for k in range(2)]
            tri = cp.tile([128, 128], MMDT["av"], tag="tri", name="tri")
            ones = cp.tile([1, 128], MMDT["v"], tag="ones")
            qk = [cp.tile([128, T], MMDT["att"], tag=f"qk{m}", name=f"qk{m}") for m in range(4)]
            v_sb = [cp.tile([128, HPG * 65], MMDT["av"], tag=f"v{m}", name=f"v{m}") for m in range(T // 128)]
            outT = [cp.tile([128, T], MMDT["proj"], tag=f"outT{k}", name=f"outT{k}") for k in range(2)]

            for k in range(KC):
                nc.sync.dma_start(xt[k][:], xt_d[128 * k:128 * (k + 1), :].bitcast(MMDT['qk']))
                nc.sync.dma_start(wqk[k][:], wqk_d[128 * k:128 * (k + 1), :].bitcast(MMDT['qk']))
                nc.sync.dma_start(wv[k][:], wv_d[128 * k:128 * (k + 1), :].bitcast(MMDT['v']))
            for m in range(4):
                nc.sync.dma_start(bqk[m][:], bqk_d[128 * m:128 * (m + 1), :])
            nc.sync.dma_start(bv[:], bv_d[:].bitcast(MMDT['v']))
            for k in range(2):
                nc.sync.dma_start(wp[k][:], wp_d[128 * k:128 * (k + 1), :].bitcast(MMDT['proj']))
            nc.sync.dma_start(tri[:], mask_d[:].bitcast(MMDT['av']))
            nc.sync.dma_start(ones[:], ones_d[:].bitcast(MMDT['v']))

            # ── stage B: qk^T [512, T] = wqk.T @ xt. k-inner over 4 PSUM
            # banks so each weight tile is loaded once per 4 matmuls. ──
            with tc.tile_pool(name="psB", bufs=8, space="PSUM") as psB:
                for mf in range(4):
                    pss = [psB.tile([128, 512], F32, tag=f"psB{nt}",
                                    name=f"psB{mf}_{nt}", bufs=1)
                           for nt in range(NT)]
                    for k in range(KC):
                        for nt in range(NT):
                            _mm(nc, pss[nt][:], wqk[k][:, 128 * mf:128 * (mf + 1)],
                                xt[k][:, 512 * nt:512 * (nt + 1)], "qk",
                                start=(k == 0), stop=(k == KC - 1))
                    for nt in range(NT):
                        nc.vector.tensor_scalar_add(
                            qk[mf][:, 512 * nt:512 * (nt + 1)], pss[nt][:],
                            bqk[mf][:])

                # ── stage C: v_aug [T, 260] = xt.T @ wv (+ ones col via K=1) ──
                for mt in range(T // 128):
                    ps = psB.tile([128, HPG * 65], F32, tag="psv", bufs=3)
                    for k in range(KC):
                        _mm(nc, ps[:], xt[k][:, 128 * mt:128 * (mt + 1)],
                            wv[k][:], "v", start=(k == 0), stop=False)
                    _mm(nc, ps[:], ones[:, :], bv[:], "k1", start=False,
                        stop=True)
                    nc.vector.tensor_copy(v_sb[mt][:], ps[:])


            # ── stage D: attention. All matmuls keep base partition 0 —
            # alternating base partitions between attT (64-row) and av
            # (128-row) matmuls measured ~1.5x slower on HW. Odd heads'
            # q/k rows live at partitions 64-127, so DMA-shift them down
            # to a base-0 scratch tile first. ──
            LAG = 5   # av lags attT by LAG units to hide the exp latency
            with (
                tc.tile_pool(name="psA", bufs=4, space="PSUM") as psA,
                tc.tile_pool(name="psAV", bufs=1, space="PSUM") as psAV,
                tc.tile_pool(name="expp", bufs=8) as expp,
                tc.tile_pool(name="recp", bufs=2) as recp,
                tc.tile_pool(name="rawp", bufs=2) as rawp,
                tc.tile_pool(name="bcp", bufs=2) as bcp,
                tc.tile_pool(name="shp", bufs=1) as shp,
            ):
                shifted = {}
                def shift_head(h):
                    # copy odd head h's q/k rows (partitions 64-127) down to 0-63
                    qtile, ktile = h // 2, 2 + h // 2
                    qs = shp.tile([64, T], MMDT["att"], tag="qs", name=f"qs{h}")
                    ks = shp.tile([64, T], MMDT["att"], tag="ks", name=f"ks{h}")
                    nc.sync.dma_start(qs[:], qk[qtile][64:128, :])
                    nc.sync.dma_start(ks[:], qk[ktile][64:128, :])
                    shifted[h] = (qs, ks)
                shift_head(1)
                for h in range(HPG):
                    if h == 1:
                        shift_head(3)
                    if h % 2 == 0:
                        qt_ap, kt_ap = qk[h // 2][0:64, :], qk[2 + h // 2][0:64, :]
                    else:
                        qs, ks = shifted.pop(h)
                        qt_ap, kt_ap = qs[:, :], ks[:, :]
                    # flat unit order grouped by k-chunk i: the attT stationary
                    # operand (k-tile) and the av stationary operand (v-tile)
                    # are each reused across the j's within a group.
                    units = [(i, j) for i in range(16) for j in range(i // 4, NT)]
                    avp = [psAV.tile([65, 512], F32, tag=f"avj{j}",
                                     name=f"avps{h}_{j}") for j in range(NT)]
                    ets = {}
                    def av_unit(u):
                        i, j = u
                        cc = 128 * (i % 4) if i == 4 * j + (i % 4) and i // 4 == j else 0
                        cc = 128 * (i % 4) if i // 4 == j else 0
                        _mm(nc, avp[j][:, cc:512],
                            v_sb[i][:, 65 * h:65 * h + 65],
                            ets.pop((i, j))[:, cc:512], "av",
                            start=(i == 0), stop=(i == 4 * j + 3))
                    for ui, (i, j) in enumerate(units):
                        diag = (i // 4 == j)
                        c0 = 128 * (i % 4) if diag else 0
                        aps = psA.tile([128, 512], F32, tag="aps",
                                       name=f"aps{h}_{j}_{i}")
                        _mm(nc, aps[:, c0:512],
                            kt_ap[:, 128 * i:128 * (i + 1)],
                            qt_ap[:, 512 * j + c0:512 * (j + 1)],
                            "att", start=True, stop=True)
                        et = expp.tile([128, 512], MMDT["av"], tag="et",
                                       name=f"et{h}_{j}_{i}")
                        nc.scalar.activation(et[:, c0:512], aps[:, c0:512],
                                             mybir.ActivationFunctionType.Exp)
                        if diag:  # triangular block at cols [c0, c0+128)
                            nc.vector.tensor_mul(et[:, c0:c0 + 128],
                                                 et[:, c0:c0 + 128], tri[:])
                        ets[(i, j)] = et
                        if ui >= LAG:
                            av_unit(units[ui - LAG])
                    for u in units[-LAG:]:
                        av_unit(u)
                    # evacuate rawout+denominator, normalize
                    for j in range(NT):
                        u = h * NT + j
                        raw = rawp.tile([65, 512], F32, tag="raw",
                                        name=f"raw{h}_{j}")
                        nc.vector.tensor_copy(raw[:], avp[j][:])
                        nc.sync.dma_start(den_d[u:u + 1, :], raw[64:65, :])
                        den2 = recp.tile([128, 4], F32, tag="den2",
                                         name=f"den2_{h}_{j}")
                        nc.sync.dma_start(
                            den2[:], bass.AP(den_d.tensor, u * 512,
                                             [[4, 128], [1, 4]]))
                        rec2 = recp.tile([128, 4], F32, tag="rec2",
                                         name=f"rec2_{h}_{j}")
                        nc.vector.reciprocal(rec2[:], den2[:])
                        nc.sync.dma_start(
                            bass.AP(rec_d.tensor, u * 512, [[4, 128], [1, 4]]),
                            rec2[:])
                        bc_sb = bcp.tile([64, 512], F32, tag="bc",
                                         name=f"bc{h}_{j}")
                        nc.sync.dma_start(
                            bc_sb[:], bass.AP(rec_d.tensor, u * 512,
                                              [[0, 64], [1, 512]]))
                        off = 64 * (h % 2)
                        nc.vector.tensor_mul(
                            outT[h // 2][off:off + 64, 512 * j:512 * (j + 1)],
                            raw[0:64, :], bc_sb[:])

            # ── stage E: y^T partial [C, T] = wp.T @ outT ──
            with (
                tc.tile_pool(name="psP", bufs=1, space="PSUM") as psP,
                tc.tile_pool(name="outp", bufs=3) as outp,
            ):
                for mo in range(8):
                    pss = [psP.tile([128, 512], F32, tag=f"psP{nt}",
                                    name=f"psP{mo}_{nt}", bufs=2)
                           for nt in range(NT)]
                    for k in range(2):
                        for nt in range(NT):
                            _mm(nc, pss[nt][:], wp[k][:, 128 * mo:128 * (mo + 1)],
                                outT[k][:, 512 * nt:512 * (nt + 1)], "proj",
                                start=(k == 0), stop=(k == 1))
                    for nt in range(NT):
                        ot = outp.tile([128, 512], F32, tag="ot",
                                       name=f"ot{mo}_{nt}", bufs=6)
                        nc.vector.tensor_copy(ot[:], pss[nt][:])
                        nc.sync.dma_start(
                            yt_d[128 * mo:128 * (mo + 1),
                                 512 * nt:512 * (nt + 1)], ot[:])

            xtp.__exit__(None, None, None)

    nc.compile()
    return nc


def _shard_inputs(x, w_qkv, b_qkv, w_proj):
    scale = 1.0 / np.sqrt(D)   # 0.125, exact power of two
    in_maps = []
    r = np.arange(128)[:, None]
    c = np.arange(128)[None, :]
    mask = np.where(c >= r, 1.0, 0.0).astype(np.float32)
    for core in range(N_CORES):
        b, g = divmod(core, HPG)
        qs = slice(CG * g, CG * (g + 1))
        ks = slice(C + CG * g, C + CG * (g + 1))
        vs = slice(2 * C + CG * g, 2 * C + CG * (g + 1))
        wqk = np.concatenate([w_qkv[qs] * scale, w_qkv[ks]], axis=0).T
        bqk = np.concatenate([b_qkv[qs] * scale, b_qkv[ks]])[:, None]
        wv_base = w_qkv[vs].T          # [C, 256]
        wv = np.zeros((C, HPG * 65), np.float32)
        bv = np.zeros((1, HPG * 65), np.float32)
        for h in range(HPG):
            wv[:, 65 * h:65 * h + 64] = wv_base[:, 64 * h:64 * h + 64]
            bv[0, 65 * h:65 * h + 64] = b_qkv[vs][64 * h:64 * h + 64]
            bv[0, 65 * h + 64] = 1.0
        in_maps.append({
            "xt": np.ascontiguousarray(x[b].T, np.float32),
            "wqk": np.ascontiguousarray(wqk, np.float32),
            "bqk": np.ascontiguousarray(bqk, np.float32),
            "wv": wv,
            "bv": bv,
            "wp": np.ascontiguousarray(w_proj[:, CG * g:CG * (g + 1)].T,
                                       np.float32),
            "mask": mask,
            "ones": np.ones((1, 128), np.float32),
        })
    return in_maps


def kernel(x, w_qkv, b_qkv, w_proj, b_proj):
    x = np.asarray(x, np.float32)
    w_qkv = np.asarray(w_qkv, np.float32)
    b_qkv = np.asarray(b_qkv, np.float32)
    w_proj = np.asarray(w_proj, np.float32)
    b_proj = np.asarray(b_proj, np.float32)

    nc = build_nc()
    in_maps = _shard_inputs(x, w_qkv, b_qkv, w_proj)
    if _trace_flag[0]:
        _ensure_ntff_hook()
    res = run_bass_kernel_spmd(nc, in_maps, core_ids=list(range(N_CORES)),
                               trace=_trace_flag[0])
    _last_results[0] = res

    y = np.empty((B, T, C), np.float32)
    for b in range(B):
        acc = np.zeros((C, T), np.float32)
        for g in range(HPG):
            acc += res.results[HPG * b + g]["yt"]
        y[b] = acc.T + b_proj[None, :]
    return y


# revision 26
# speedup vs baseline: 1.2947x; 1.2562x over previous
"""Causal self-attention (B=2, T=2048, C=1024, H=16, D=64) on 8 trn2 cores.

Sharding: core c -> batch b = c // 4, head-group g = c % 4 (4 heads each).
Data-parallel over B, tensor-parallel (Megatron) over heads for the
qkv / proj linears. Each core computes its head-group's attention and a
partial output projection; the host sums the 4 partials per batch and
adds the proj bias.

Everything on-device is done in transposed [feature, token] space:
  qk^T = Wqk @ x^T                       (PE; bias added by ACT on evacuate)
  v    = x @ Wv^T (+ ones column)        (PE; K=1 matmul adds bias + ones)
  att^T[k, q] = k^T(head)^T . q(head)    (PE, K=64; causal tiles skipped)
  P = exp(att^T + additive causal mask)  (ACT; no max-subtraction needed,
                                          |logits| <~ 10 so fp32 exp is safe)
  rawout^T[d+1, q] = v_aug^T @ P         (PE accumulate over k chunks; the
                                          ones column makes row 64 = sum_k P
                                          = softmax denominator, for free)
  out^T = rawout^T[:64] * (1/denom)      (DVE recip + PE K=1 broadcast + DVE)
  y^T(partial) = Wp_g^T.T @ out^T        (PE)
"""

import os
import sys
import types

for _p in ("/opt/trn_rl_repo", "/root/.axon_site", "/root/.axon_site/_ro/trn_rl_repo"):
    if os.path.isdir(_p) and _p not in sys.path:
        sys.path.append(_p)

import numpy as np

import concourse.bacc as bacc
import concourse.bass as bass
import concourse.mybir as mybir
import concourse.tile as tile
from concourse.bass_utils import run_bass_kernel_spmd

# ── problem constants (hardcoded; spec.json not available at grade time) ──
B, T, C = 2, 2048, 1024
H, D = 16, 64
N_CORES = 8
HPG = 4                 # heads per group (per core)
CG = HPG * D            # 256 channels per head-group
NT = T // 512           # 4 token chunks of 512
KC = C // 128           # 8 contraction tiles for C
NEG = -1.0e4            # (unused) additive mask value
VW = HPG * 65 + 64      # v tile width: 4x(64+ones col) + 64 zero-pad cols

F32 = mybir.dt.float32
F32R = mybir.dt.float32r
# per-stage matmul operand dtype: float32 (exact, 4 cyc/row) or float32r
# (tf32-like, 1 cyc/row at N>=256). Overridable for A/B testing.
MMDT = {
    "qk": F32R, "v": F32R, "att": F32R, "av": F32R, "proj": F32R, "k1": F32,
}
_trace_flag = [False]   # test.py can flip this to capture a profile
_last_results = [None]


def _mm(nc, out, lhsT, rhs, stage, **kw):
    nc.tensor.matmul(out, lhsT, rhs, **kw)


def _ensure_ntff_hook():
    """Install the NTFF profile hook shim (container's antenv lacks it)."""
    if "antenv.axon_hooks" in sys.modules:
        return
    try:
        from trn_agent_boot.trn_boot import _ntff_profile_via_ctypes
    except Exception:
        return
    mod = types.ModuleType("antenv.axon_hooks")
    hook = [None]
    mod.set_axon_ntff_profile_hook = lambda h: hook.__setitem__(0, h)
    mod.get_axon_ntff_profile_hook = lambda: hook[0]
    sys.modules["antenv.axon_hooks"] = mod
    so = "/opt/axon/libaxon_pjrt.so"
    if os.path.exists(so):
        mod.set_axon_ntff_profile_hook(_ntff_profile_via_ctypes(so))


def build_nc():
    nc = bacc.Bacc("TRN2", target_bir_lowering=False, debug=False,
                   num_devices=N_CORES)

    xt_d = nc.dram_tensor("xt", [C, T], F32, kind="ExternalInput").ap()
    wqk_d = nc.dram_tensor("wqk", [C, 2 * CG], F32, kind="ExternalInput").ap()
    bqk_d = nc.dram_tensor("bqk", [2 * CG, 1], F32, kind="ExternalInput").ap()
    wv_d = nc.dram_tensor("wv", [C, VW], F32, kind="ExternalInput").ap()
    bv_d = nc.dram_tensor("bv", [1, VW], F32, kind="ExternalInput").ap()
    wp_d = nc.dram_tensor("wp", [CG, C], F32, kind="ExternalInput").ap()
    mask_d = nc.dram_tensor("mask", [128, 128], F32, kind="ExternalInput").ap()
    ones_d = nc.dram_tensor("ones", [1, 128], F32, kind="ExternalInput").ap()
    zeros_d = nc.dram_tensor("zeros", [64, T], F32, kind="ExternalInput").ap()
    yt_d = nc.dram_tensor("yt", [C, T], F32, kind="ExternalOutput").ap()
    rec_d = nc.dram_tensor("rec_scratch", [HPG * NT, 512], F32).ap()
    den_d = nc.dram_tensor("den_scratch", [HPG * NT, 512], F32).ap()

    with tile.TileContext(nc) as tc:
        with tc.tile_pool(name="const", bufs=1) as cp:
            # ── persistent SBUF residents ──
            assert MMDT["v"] == MMDT["qk"]
            xtp = tc.tile_pool(name="xtp", bufs=1)
            xtpool = xtp.__enter__()
            xt = [xtpool.tile([128, T], MMDT["qk"], tag=f"xt{k}", name=f"xt{k}") for k in range(KC)]
            wqk = [cp.tile([128, 2 * CG], MMDT["qk"], tag=f"wqk{k}", name=f"wqk{k}") for k in range(KC)]
            wv = [cp.tile([128, VW], MMDT["v"], tag=f"wv{k}", name=f"wv{k}") for k in range(KC)]
            bqk = [cp.tile([128, 1], F32, tag=f"bqk{m}", name=f"bqk{m}") for m in range(4)]
            bv = cp.tile([1, VW], MMDT["v"], tag="bv")
            wp = [cp.tile([128, C], MMDT["proj"], tag=f"wp{k}", name=f"wp{k}") for k in range(2)]
            tri = cp.tile([128, 128], MMDT["av"], tag="tri", name="tri")
            ones = cp.tile([1, 128], MMDT["v"], tag="ones")
            qk = [cp.tile([128, T], MMDT["att"], tag=f"qk{m}", name=f"qk{m}") for m in range(4)]
            v_sb = [cp.tile([128, VW], MMDT["av"], tag=f"v{m}", name=f"v{m}") for m in range(T // 128)]
            outT = [cp.tile([128, T], MMDT["proj"], tag=f"outT{k}", name=f"outT{k}") for k in range(2)]

            for k in range(KC):
                nc.sync.dma_start(xt[k][:], xt_d[128 * k:128 * (k + 1), :].bitcast(MMDT['qk']))
                nc.sync.dma_start(wqk[k][:], wqk_d[128 * k:128 * (k + 1), :].bitcast(MMDT['qk']))
                nc.sync.dma_start(wv[k][:], wv_d[128 * k:128 * (k + 1), :].bitcast(MMDT['v']))
            for m in range(4):
                nc.sync.dma_start(bqk[m][:], bqk_d[128 * m:128 * (m + 1), :])
            nc.sync.dma_start(bv[:], bv_d[:].bitcast(MMDT['v']))
            for k in range(2):
                nc.sync.dma_start(wp[k][:], wp_d[128 * k:128 * (k + 1), :].bitcast(MMDT['proj']))
            nc.sync.dma_start(tri[:], mask_d[:].bitcast(MMDT['av']))
            nc.sync.dma_start(ones[:], ones_d[:].bitcast(MMDT['v']))

            # ── stage B: qk^T [512, T] = wqk.T @ xt. k-inner over 4 PSUM
            # banks so each weight tile is loaded once per 4 matmuls. ──
            with tc.tile_pool(name="psB", bufs=8, space="PSUM") as psB:
                for mf in range(4):
                    pss = [psB.tile([128, 512], F32, tag=f"psB{nt}",
                                    name=f"psB{mf}_{nt}", bufs=1)
                           for nt in range(NT)]
                    for k in range(KC):
                        for nt in range(NT):
                            _mm(nc, pss[nt][:], wqk[k][:, 128 * mf:128 * (mf + 1)],
                                xt[k][:, 512 * nt:512 * (nt + 1)], "qk",
                                start=(k == 0), stop=(k == KC - 1))
                    for nt in range(NT):
                        nc.vector.tensor_scalar_add(
                            qk[mf][:, 512 * nt:512 * (nt + 1)], pss[nt][:],
                            bqk[mf][:])

                # ── stage C: v_aug [T, 260] = xt.T @ wv (+ ones col via K=1) ──
                for mt in range(T // 128):
                    ps = psB.tile([128, VW], F32, tag="psv", bufs=3)
                    for k in range(KC):
                        _mm(nc, ps[:], xt[k][:, 128 * mt:128 * (mt + 1)],
                            wv[k][:], "v", start=(k == 0), stop=False)
                    _mm(nc, ps[:], ones[:, :], bv[:], "k1", start=False,
                        stop=True)
                    nc.vector.tensor_copy(v_sb[mt][:], ps[:])


            # ── stage D: attention. All matmuls keep base partition 0 —
            # alternating base partitions between attT (64-row) and av
            # (128-row) matmuls measured ~1.5x slower on HW. Odd heads'
            # q/k rows live at partitions 64-127, so DMA-shift them down
            # to a base-0 scratch tile first. ──
            LAG = 5   # av lags attT by LAG units to hide the exp latency
            with (
                tc.tile_pool(name="psA", bufs=4, space="PSUM") as psA,
                tc.tile_pool(name="psAV", bufs=1, space="PSUM") as psAV,
                tc.tile_pool(name="expp", bufs=7) as expp,
                tc.tile_pool(name="recp", bufs=2) as recp,
                tc.tile_pool(name="rawp", bufs=2) as rawp,
                tc.tile_pool(name="bcp", bufs=2) as bcp,
                tc.tile_pool(name="shp", bufs=1) as shp,
            ):
                shifted = {}
                qs = shp.tile([128, T], MMDT["att"], tag="qs", name="qs")
                ks = shp.tile([128, T], MMDT["att"], tag="ks", name="ks")
                # zero the contraction-pad rows once; per-head DMAs only
                # rewrite rows 0-63, so the zeros persist across heads
                nc.sync.dma_start(qs[64:128, :], zeros_d[:].bitcast(MMDT["att"]))
                nc.sync.dma_start(ks[64:128, :], zeros_d[:].bitcast(MMDT["att"]))
                for h in range(HPG):
                    qtile, ktile = h // 2, 2 + h // 2
                    off = 64 * (h % 2)
                    nc.sync.dma_start(qs[0:64, :], qk[qtile][off:off + 64, :])
                    nc.sync.dma_start(ks[0:64, :], qk[ktile][off:off + 64, :])
                    qt_ap, kt_ap = qs[:, :], ks[:, :]
                    # flat unit order grouped by k-chunk i: the attT stationary
                    # operand (k-tile) and the av stationary operand (v-tile)
                    # are each reused across the j's within a group.
                    units = [(i, j) for i in range(16) for j in range(i // 4, NT)]
                    avp = [psAV.tile([128, 512], F32, tag=f"avj{j}",
                                     name=f"avps{h}_{j}") for j in range(NT)]
                    ets = {}
                    def av_unit(u):
                        i, j = u
                        cc = 128 * (i % 4) if i == 4 * j + (i % 4) and i // 4 == j else 0
                        cc = 128 * (i % 4) if i // 4 == j else 0
                        _mm(nc, avp[j][:, cc:512],
                            v_sb[i][:, 65 * h:65 * h + 128],
                            ets.pop((i, j))[:, cc:512], "av",
                            start=(i == 0), stop=(i == 4 * j + 3))
                    for ui, (i, j) in enumerate(units):
                        diag = (i // 4 == j)
                        c0 = 128 * (i % 4) if diag else 0
                        aps = psA.tile([128, 512], F32, tag="aps",
                                       name=f"aps{h}_{j}_{i}")
                        _mm(nc, aps[:, c0:512],
                            kt_ap[:, 128 * i:128 * (i + 1)],
                            qt_ap[:, 512 * j + c0:512 * (j + 1)],
                            "att", start=True, stop=True)
                        et = expp.tile([128, 512], MMDT["av"], tag="et",
                                       name=f"et{h}_{j}_{i}")
                        nc.scalar.activation(et[:, c0:512], aps[:, c0:512],
                                             mybir.ActivationFunctionType.Exp)
                        if diag:  # triangular block at cols [c0, c0+128)
                            nc.vector.tensor_mul(et[:, c0:c0 + 128],
                                                 et[:, c0:c0 + 128], tri[:])
                        ets[(i, j)] = et
                        if ui >= LAG:
                            av_unit(units[ui - LAG])
                    for u in units[-LAG:]:
                        av_unit(u)
                    # evacuate rawout+denominator, normalize
                    for j in range(NT):
                        u = h * NT + j
                        raw = rawp.tile([65, 512], F32, tag="raw",
                                        name=f"raw{h}_{j}")
                        nc.vector.tensor_copy(raw[:], avp[j][0:65, :])
                        nc.sync.dma_start(den_d[u:u + 1, :], raw[64:65, :])
                        den2 = recp.tile([128, 4], F32, tag="den2",
                                         name=f"den2_{h}_{j}")
                        nc.sync.dma_start(
                            den2[:], bass.AP(den_d.tensor, u * 512,
                                             [[4, 128], [1, 4]]))
                        rec2 = recp.tile([128, 4], F32, tag="rec2",
                                         name=f"rec2_{h}_{j}")
                        nc.vector.reciprocal(rec2[:], den2[:])
                        nc.sync.dma_start(
                            bass.AP(rec_d.tensor, u * 512, [[4, 128], [1, 4]]),
                            rec2[:])
                        bc_sb = bcp.tile([64, 512], F32, tag="bc",
                                         name=f"bc{h}_{j}")
                        nc.sync.dma_start(
                            bc_sb[:], bass.AP(rec_d.tensor, u * 512,
                                              [[0, 64], [1, 512]]))
                        off = 64 * (h % 2)
                        nc.vector.tensor_mul(
                            outT[h // 2][off:off + 64, 512 * j:512 * (j + 1)],
                            raw[0:64, :], bc_sb[:])

            # ── stage E: y^T partial [C, T] = wp.T @ outT ──
            with (
                tc.tile_pool(name="psP", bufs=1, space="PSUM") as psP,
                tc.tile_pool(name="outp", bufs=3) as outp,
            ):
                for mo in range(8):
                    pss = [psP.tile([128, 512], F32, tag=f"psP{nt}",
                                    name=f"psP{mo}_{nt}", bufs=2)
                           for nt in range(NT)]
                    for k in range(2):
                        for nt in range(NT):
                            _mm(nc, pss[nt][:], wp[k][:, 128 * mo:128 * (mo + 1)],
                                outT[k][:, 512 * nt:512 * (nt + 1)], "proj",
                                start=(k == 0), stop=(k == 1))
                    for nt in range(NT):
                        ot = outp.tile([128, 512], F32, tag="ot",
                                       name=f"ot{mo}_{nt}", bufs=4)
                        nc.vector.tensor_copy(ot[:], pss[nt][:])
                        nc.sync.dma_start(
                            yt_d[128 * mo:128 * (mo + 1),
                                 512 * nt:512 * (nt + 1)], ot[:])

            xtp.__exit__(None, None, None)

    nc.compile()
    return nc


def _shard_inputs(x, w_qkv, b_qkv, w_proj):
    scale = 1.0 / np.sqrt(D)   # 0.125, exact power of two
    in_maps = []
    r = np.arange(128)[:, None]
    c = np.arange(128)[None, :]
    mask = np.where(c >= r, 1.0, 0.0).astype(np.float32)
    for core in range(N_CORES):
        b, g = divmod(core, HPG)
        qs = slice(CG * g, CG * (g + 1))
        ks = slice(C + CG * g, C + CG * (g + 1))
        vs = slice(2 * C + CG * g, 2 * C + CG * (g + 1))
        wqk = np.concatenate([w_qkv[qs] * scale, w_qkv[ks]], axis=0).T
        bqk = np.concatenate([b_qkv[qs] * scale, b_qkv[ks]])[:, None]
        wv_base = w_qkv[vs].T          # [C, 256]
        wv = np.zeros((C, VW), np.float32)
        bv = np.zeros((1, VW), np.float32)
        for h in range(HPG):
            wv[:, 65 * h:65 * h + 64] = wv_base[:, 64 * h:64 * h + 64]
            bv[0, 65 * h:65 * h + 64] = b_qkv[vs][64 * h:64 * h + 64]
            bv[0, 65 * h + 64] = 1.0
        in_maps.append({
            "xt": np.ascontiguousarray(x[b].T, np.float32),
            "wqk": np.ascontiguousarray(wqk, np.float32),
            "bqk": np.ascontiguousarray(bqk, np.float32),
            "wv": wv,
            "bv": bv,
            "wp": np.ascontiguousarray(w_proj[:, CG * g:CG * (g + 1)].T,
                                       np.float32),
            "mask": mask,
            "ones": np.ones((1, 128), np.float32),
            "zeros": np.zeros((64, T), np.float32),
        })
    return in_maps


def kernel(x, w_qkv, b_qkv, w_proj, b_proj):
    x = np.asarray(x, np.float32)
    w_qkv = np.asarray(w_qkv, np.float32)
    b_qkv = np.asarray(b_qkv, np.float32)
    w_proj = np.asarray(w_proj, np.float32)
    b_proj = np.asarray(b_proj, np.float32)

    nc = build_nc()
    in_maps = _shard_inputs(x, w_qkv, b_qkv, w_proj)
    if _trace_flag[0]:
        _ensure_ntff_hook()
    res = run_bass_kernel_spmd(nc, in_maps, core_ids=list(range(N_CORES)),
                               trace=_trace_flag[0])
    _last_results[0] = res

    y = np.empty((B, T, C), np.float32)
    for b in range(B):
        acc = np.zeros((C, T), np.float32)
        for g in range(HPG):
            acc += res.results[HPG * b + g]["yt"]
        y[b] = acc.T + b_proj[None, :]
    return y
